# revision 11
# baseline (speedup 1.0000x reference)
"""Cost-sensitive focal NLL loss on 8 Trainium2 NeuronCores.

For feature [N, C] logits and label [N] int:
    log_p = log_softmax(feature, axis=1)
    p = exp(log_p); beta = (1 - p)**2
    counts = bincount(label, C); ni = counts[label]; r = ni / N
    alpha = exp(r - 1) / r
    loss = -mean(alpha * beta[i, label[i]] * log_p[i, label[i]])

Only the O(N*C) softmax statistics need the device: per row we need
s = sum_c exp(x_c) and the raw logit x_label.  Everything derived from
the labels alone (global class histogram -> per-row alpha, and the
flat gather index 1000*t + label) is O(N) input preprocessing done
exactly on the host, like the label layout transform.

The device program is raw bass (no TileContext): Tile's end-of-kernel
drain + semaphore-clear + double butterfly barrier costs ~8.5us of
serial EVENT_SEMAPHORE churn, and a single-shot loss kernel doesn't
need recyclable semaphores.  Raw-mode lessons baked in here:
  - HWDGE completion sems: one per DMA, +16 when the transfer landed.
  - The ScalarE accumulator drain retires asynchronously even w.r.t.
    later same-engine instructions, so row sums are NOT accum_out:
    VectorE (idle during the stream) does per-tile tensor_reduce
    instead, which also takes ~190ns/tile of READ_ACCUMULATOR off the
    ScalarE critical path.
  - Back-to-back dependent DVE ops overlap in the pipe with no RAW
    interlock; an explicit drain() between dependent pairs makes
    writes visible.  Cross-engine sem updates already imply
    visibility.
  - indirect_copy needs the standard GPSIMD ucode library loaded
    explicitly (insert_library_loads does not track it).
  - A hand-emitted LoadActFuncSet(natural_log_exp_and_others) before
    the stream gives exp AND ln in one table set - no mid-kernel
    table switch at all.

Per core: tile 0 in two column halves then 15 whole row-tile DMAs
[128,1000] land in one 62.5KB/partition SBUF block; ScalarE streams
pure exp; VectorE reduces each exp image to the row sums; GpSimd picks
x_label out of the raw shard with one indirect_copy; the [128,16] tail
(xe, ln, then p/beta/alpha math) folds through a ones-vector matmul to
[1,16] PSUM, reduces to [1,1], and ships 4 bytes.  Host sums 8 scalars
and divides by -N.
"""

import os

import numpy as np

import concourse.bacc as bacc
import concourse.bass as bass
import concourse.mybir as mybir
from concourse.bass_utils import run_bass_kernel_spmd

N_CORES = 8
N = 16384
C = 1000
P = 128
ROWS = N // N_CORES          # 2048 rows per core
T = ROWS // P                # 16 row-tiles per core

FP = mybir.dt.float32
U16 = mybir.dt.uint16

NAT_LOG_EXP_SET = 6          # act_info.json: natural_log_exp_and_others

LAST_RESULTS = None  # BassKernelResults of the most recent run (for profiling)


def build_program():
    nc = bacc.Bacc(
        "TRN2",
        target_bir_lowering=False,
        debug=False,
        enable_asserts=False,
        num_devices=N_CORES,
    )

    feature = nc.dram_tensor("feature", [ROWS, C], FP, kind="ExternalInput")
    # gidx[p, t] = 1000*t + label[128*t + p]: flat gather index into the
    # on-chip shard image (host-computed)
    gidx_in = nc.dram_tensor("gidx", [P, T], U16, kind="ExternalInput")
    # alpha[p, t] = exp(r-1)/r for row 128*t + p, from the exact global
    # bincount (host-computed)
    alpha_in = nc.dram_tensor("alpha", [P, T], FP, kind="ExternalInput")
    out = nc.dram_tensor("out", [1, 1], FP, kind="ExternalOutput")
    dbg = {}
    if bool(int(os.environ.get("KERNEL_DEBUG", "0"))):
        for nm in ["d_xl", "d_scol", "d_xe", "d_lns", "d_u", "d_gidx"]:
            dt = U16 if nm == "d_gidx" else FP
            dbg[nm] = nc.dram_tensor(nm, [P, T], dt, kind="ExternalOutput")

    ftall = nc.alloc_sbuf_tensor("ftall", [P, T * C], FP)
    esbufs = [nc.alloc_sbuf_tensor(f"es{t}", [P, C], FP) for t in range(T)]
    gidx = nc.alloc_sbuf_tensor("gidx_sb", [P, T], U16)
    alpha = nc.alloc_sbuf_tensor("alpha_sb", [P, T], FP)
    s_col = nc.alloc_sbuf_tensor("s_col", [P, T], FP)
    xl = nc.alloc_sbuf_tensor("xl", [P, T], FP)
    xe = nc.alloc_sbuf_tensor("xe", [P, T], FP)
    ln_s = nc.alloc_sbuf_tensor("ln_s", [P, T], FP)
    sinv = nc.alloc_sbuf_tensor("sinv", [P, T], FP)
    pp = nc.alloc_sbuf_tensor("pp", [P, T], FP)
    logp = nc.alloc_sbuf_tensor("logp", [P, T], FP)
    pm1 = nc.alloc_sbuf_tensor("pm1", [P, T], FP)
    beta = nc.alloc_sbuf_tensor("beta", [P, T], FP)
    aw = nc.alloc_sbuf_tensor("aw", [P, T], FP)
    u = nc.alloc_sbuf_tensor("u", [P, T], FP)
    ones_col = nc.alloc_sbuf_tensor("ones_col", [P, 1], FP)
    fin = nc.alloc_sbuf_tensor("fin", [1, 1], FP)
    colsum = nc.alloc_psum_tensor("colsum", [1, T], FP)

    H = C // 2  # tile-0 column halves so the first exp starts early

    from contextlib import ExitStack

    with ExitStack() as ctx:
        block = ctx.enter_context(nc.Block())
        qd = [ctx.enter_context(nc.semaphore(f"qd{i}")) for i in range(T)]
        sw_gidx = ctx.enter_context(nc.semaphore("sw_gidx"))
        sw_alpha = ctx.enter_context(nc.semaphore("sw_alpha"))
        pool_done = ctx.enter_context(nc.semaphore("pool_done"))
        act_done = ctx.enter_context(nc.semaphore("act_done"))
        exp_done = ctx.enter_context(nc.semaphore("exp_done"))
        red_done = ctx.enter_context(nc.semaphore("red_done"))
        dve_done = ctx.enter_context(nc.semaphore("dve_done"))
        pe_done = ctx.enter_context(nc.semaphore("pe_done"))
        out_done = ctx.enter_context(nc.semaphore("out_done"))

        # tile 0's two column halves both bump qd[0] (tile complete at
        # >=32); tile t>=1 complete at qd[t] >= 16

        @block.sync
        def _(sync):
            for s in range(2):
                sync.dma_start(
                    ftall[:, s * H : (s + 1) * H],
                    feature.ap()[0:P, s * H : (s + 1) * H],
                ).then_inc(qd[0], 16)
            for t in range(1, T):
                sync.dma_start(
                    ftall[:, t * C : (t + 1) * C],
                    feature.ap()[t * P : (t + 1) * P, :],
                ).then_inc(qd[t], 16)
            sync.wait_ge(dve_done, 2)
            sync.dma_start(out.ap(), fin[:]).then_inc(out_done, 16)
            nout = 1
            if dbg:
                for nm, sb in [("d_xl", xl), ("d_scol", s_col), ("d_xe", xe),
                               ("d_lns", ln_s), ("d_u", u), ("d_gidx", gidx)]:
                    sync.dma_start(dbg[nm].ap(), sb[:]).then_inc(out_done, 16)
                    nout += 1
            sync.wait_ge(out_done, 16 * nout)

        @block.gpsimd
        def _(gpsimd):
            # indirect_copy needs the standard GPSIMD ucode library resident,
            # but insert_library_loads doesn't track InstIndirectCopy --
            # load it explicitly (early, overlapped with the stream)
            from concourse import library_config

            gpsimd.load_library(library_config.standard)
            gpsimd.dma_start(gidx[:], gidx_in.ap()).then_inc(sw_gidx, 16)
            gpsimd.dma_start(alpha[:], alpha_in.ap()).then_inc(sw_alpha, 16)
            gpsimd.wait_ge(sw_gidx, 16)
            gpsimd.wait_ge(qd[0], 32)
            for t in range(1, T):
                gpsimd.wait_ge(qd[t], 16)
            gpsimd.indirect_copy(
                xl[:], ftall[:], gidx[:],
                i_know_ap_gather_is_preferred=True,
            ).then_inc(pool_done)

        @block.scalar
        def _(scalar):
            # one resident table set with BOTH exp and ln: no switch later.
            # insert_act_table_loads' fixpoint adopts pre-placed loads.
            ld = mybir.InstLoadActFuncSet(
                name=nc.get_next_instruction_name(), ins=[], outs=[],
                act_func_set_id=NAT_LOG_EXP_SET,
            )
            ld.engine = scalar.engine
            scalar.add_instruction(ld)
            for s in range(2):
                scalar.wait_ge(qd[0], 16 * (s + 1))
                scalar.activation(
                    esbufs[0][:, s * H : (s + 1) * H],
                    ftall[:, s * H : (s + 1) * H],
                    mybir.ActivationFunctionType.Exp,
                ).then_inc(exp_done)
            for t in range(1, T):
                scalar.wait_ge(qd[t], 16)
                scalar.activation(
                    esbufs[t][:],
                    ftall[:, t * C : (t + 1) * C],
                    mybir.ActivationFunctionType.Exp,
                ).then_inc(exp_done)
            scalar.wait_ge(pool_done, 1)
            scalar.activation(
                xe[:], xl[:], mybir.ActivationFunctionType.Exp
            ).then_inc(act_done)
            scalar.wait_ge(red_done, T)
            scalar.activation(
                ln_s[:], s_col[:], mybir.ActivationFunctionType.Ln
            ).then_inc(act_done)

        @block.vector
        def _(vector):
            vector.memset(ones_col[:], 1.0)
            vector.wait_ge(sw_alpha, 16)
            # per-tile row sums off the exp images (disjoint writes, no
            # intra-chain hazards -> no drains); tile 0 needs both halves
            for t in range(T):
                vector.wait_ge(exp_done, t + 2)
                vector.tensor_reduce(
                    s_col[:, t : t + 1], esbufs[t][:],
                    axis=mybir.AxisListType.X, op=mybir.AluOpType.add,
                ).then_inc(red_done)
            # tail: act_done>=2 implies ln/xe writes visible (cross-engine
            # sem updates fire after the writes land), and transitively the
            # reduces' writes too.  Same-engine dependent pairs still need
            # an explicit drain.
            vector.wait_ge(act_done, 2)
            vector.reciprocal(sinv[:], s_col[:])
            vector.tensor_tensor(logp[:], xl[:], ln_s[:],
                                 op=mybir.AluOpType.subtract)
            vector.drain()
            vector.tensor_tensor(pp[:], xe[:], sinv[:],
                                 op=mybir.AluOpType.mult)
            vector.tensor_tensor(aw[:], alpha[:], logp[:],
                                 op=mybir.AluOpType.mult)
            vector.drain()
            vector.tensor_scalar(pm1[:], pp[:], 1.0, None,
                                 op0=mybir.AluOpType.subtract)
            vector.drain()
            vector.tensor_tensor(beta[:], pm1[:], pm1[:],
                                 op=mybir.AluOpType.mult)
            vector.drain()
            vector.tensor_tensor(u[:], beta[:], aw[:],
                                 op=mybir.AluOpType.mult).then_inc(dve_done)
            vector.wait_ge(pe_done, 1)
            vector.tensor_reduce(
                fin[:], colsum[:], axis=mybir.AxisListType.X,
                op=mybir.AluOpType.add,
            ).then_inc(dve_done)

        @block.tensor
        def _(tensor):
            tensor.wait_ge(dve_done, 1)
            tensor.matmul(colsum[:], lhsT=ones_col[:], rhs=u[:],
                          start=True, stop=True).then_inc(pe_done)

    nc.compile()
    return nc


_NC_CACHE = None


def _get_nc():
    global _NC_CACHE
    if _NC_CACHE is None:
        _NC_CACHE = build_program()
    return _NC_CACHE


def kernel(feature: np.ndarray, label: np.ndarray) -> np.ndarray:
    global LAST_RESULTS
    feature = np.ascontiguousarray(np.asarray(feature, dtype=np.float32))
    label = np.asarray(label)
    assert feature.shape == (N, C), feature.shape
    assert label.shape == (N,), label.shape

    lab64 = label.astype(np.int64)
    counts = np.bincount(lab64, minlength=C).astype(np.float64)
    ni = counts[lab64]                      # [N]
    r = ni / N
    alpha = (np.exp(r - 1.0) / r).astype(np.float32)
    # flat on-chip gather index: row 128*t + p of the shard sits at
    # ftall[p, 1000*t + c]
    tbase = (np.arange(T, dtype=np.uint16) * C)[None, :]  # [1, T]

    in_maps = []
    for k in range(N_CORES):
        sl = slice(k * ROWS, (k + 1) * ROWS)
        lab_pt = label[sl].astype(np.uint16).reshape(T, P).T  # [p, t]
        alpha_pt = np.ascontiguousarray(alpha[sl].reshape(T, P).T)
        gidx = np.ascontiguousarray(lab_pt + tbase)
        in_maps.append(
            {
                "feature": np.ascontiguousarray(feature[sl]),
                "gidx": gidx,
                "alpha": alpha_pt,
            }
        )

    nc = _get_nc()
    trace = bool(int(os.environ.get("KERNEL_TRACE", "0")))
    res = run_bass_kernel_spmd(
        nc,
        in_maps,
        core_ids=list(range(N_CORES)),
        trace=trace,
    )
    LAST_RESULTS = res

    total = 0.0
    for k in range(N_CORES):
        total += float(res.results[k]["out"][0, 0])
    return np.float32(-total / N)
